# revision 33
# baseline (speedup 1.0000x reference)
"""Trainium2 Bass kernel for nn_DepthAttention (depth attention over d=32).

Reference computation (per pixel (b,h,w), all 1x1 convs):
  q = Wq x               [320]   (heads=8 x dh=40)
  k = Wk ctx[:, d]       [320, 32]
  v = Wv ctx[:, d]       [320, 32]
  sim[n,d] = sum_{c in head n} q[c] k[c,d] * scale
  attn = softmax_d(sim)
  o[c] = sum_d v[c,d] attn[head(c),d]
  y = Wout o + bout      [320]

Sharding: h (64) split across 8 cores -> 8 rows of h per core, no halo.
Per core: 1024 pixels in 8 blocks of P=128. The host pre-casts
context/x to bf16 and lays them out as per-block contiguous panels
(one 8 KiB DMA run per channel row). The context panel carries a 321st
constant-ones channel: the v-projection's chunk-2 matmul uses it to
emit softmax-denominator rows for free.

The block loop is software-pipelined depth 3: phase A (DMA, q/k
projections, k*q, selector-reduce to sim[8 rows]) of block i is
emitted before phase B1 (broadcast+exp, v projection, v*attn,
d-reduce) of block i-1 and phase B2 (reciprocal, normalize, output
projection, DMA out) of block i-2, so the PE's in-order stream never
waits on the cross-engine softmax chain (which kept re-triggering the
HAM clock throttle in the depth-2 version).

Free-dim layout is pixel-major (p, d) — d innermost — so the d-reduce
on DVE reads stride-1.

Engine mapping:
  PE : all matmuls in bf16 (1 cyc/row, FWL weight loads): q/k/v
       projections, 320->8 selector reduce of k*q, 8->320 broadcast of
       sim, 1/den broadcast, Wout.
  DVE: k*q (q broadcast over d via step-0 AP), v*attn, reduce over d
       (d innermost via pixel-major v layout), reciprocal, bias add.
  ACT: psum->sbuf drains; exp AFTER broadcast on 128-row tiles.
       Softmax max-subtraction is skipped (logits are O(1) here).
"""

import sys

sys.path.insert(0, "/opt/trn_rl_repo")

from contextlib import ExitStack  # noqa: E402

import ml_dtypes  # noqa: E402
import numpy as np  # noqa: E402

import concourse.bacc as bacc  # noqa: E402
import concourse.bass as bass  # noqa: E402
import concourse.mybir as mybir  # noqa: E402
import concourse.tile as tile  # noqa: E402

HEADS = 8
DH = 40
CIN = 320
INNER = HEADS * DH  # 320
D = 32
B = 2
H = 64
W = 64
NCORES = 8
HLOC = H // NCORES  # 8
PIX_B = HLOC * W  # 512
P = 128
NBLK = B * PIX_B // P  # 8
NT = (D * P) // 512  # 8
SCALE = DH ** -0.5

F32 = mybir.dt.float32
F32R = mybir.dt.float32r
BF16 = mybir.dt.bfloat16
NPBF = ml_dtypes.bfloat16

CHUNKS = [(0, 128), (128, 128), (256, 64)]
# v-projection output sizes: chunk2 carries 8 extra denominator rows
VSZ = [128, 128, 72]


def _head_of(c):
    return c // DH


def _bcast_runs(o0, nv):
    """Row-replication runs for broadcasting attn head-rows into a chunk's
    channel rows: list of (row0, head0, n_heads, reps_per_head)."""
    runs, r = [], 0
    while r < nv:
        c = o0 + r
        h = c // DH
        run = min((h + 1) * DH - c, nv - r)
        if run == DH:
            nh = 1
            while (r + (nh + 1) * DH <= nv and (o0 + r + nh * DH) % DH == 0):
                nh += 1
            runs.append((r, h, nh, DH))
            r += nh * DH
        else:
            runs.append((r, h, 1, run))
            r += run
    return runs


def make_constants():
    sel = np.zeros((128, 32), NPBF)
    for kc, (c0, csz) in enumerate(CHUNKS):
        for r in range(csz):
            sel[r, kc * 8 + _head_of(c0 + r)] = 1.0
    for r in range(64):  # chunk2 selector replica at partition base 64
        sel[64 + r, 24 + _head_of(256 + r)] = 1.0
    bsel = np.zeros((8, 384), NPBF)
    for mo, (c0, csz) in enumerate(CHUNKS):
        for r in range(csz):
            bsel[_head_of(c0 + r), mo * 128 + r] = 1.0
    for n in range(8):
        bsel[n, 2 * 128 + 64 + n] = 1.0
    rsel = np.zeros((8, 384), np.float32)
    for mo, (c0, csz) in enumerate(CHUNKS):
        for r in range(csz):
            rsel[_head_of(c0 + r), mo * 128 + r] = 1.0
    return sel, bsel, rsel


def pack_weight_T(w, ones_cols=False):
    """w [out, in] -> bf16 packed lhsT [128, 3*M] with M = out (+8 den
    cols when ones_cols).  Chunk kc of the 'in' dim at free offset kc*M;
    chunk 2 gets an extra contraction row 64 (the ones-channel), wired to
    the 8 denominator columns when ones_cols."""
    wt = np.ascontiguousarray(w.T, dtype=np.float32)  # [in, out]
    od = wt.shape[1]
    m = od + 8 if ones_cols else od
    p = np.zeros((128, 3 * m), NPBF)
    for kc, (c0, csz) in enumerate(CHUNKS):
        p[0:csz, kc * m:kc * m + od] = wt[c0:c0 + csz, :]
    if ones_cols:
        for n in range(8):
            p[64, 2 * m + od + n] = 1.0  # ones-channel -> den col n (chunk2)
    return p


def build_nc():
    nc = bacc.Bacc(
        "TRN2",
        target_bir_lowering=False,
        debug=False,
        enable_asserts=False,
        num_devices=NCORES,
    )

    ctx_t = nc.dram_tensor("ctx", [NBLK, CIN + 1, D * P], BF16, kind="ExternalInput")
    s8d_t = nc.dram_tensor("s8d", [2, 8, D * P], BF16, kind="Internal")
    x_t = nc.dram_tensor("x", [NBLK, CIN, P], BF16, kind="ExternalInput")
    wq_t = nc.dram_tensor("wq_p", [128, 960], BF16, kind="ExternalInput")
    wk_t = nc.dram_tensor("wk_p", [128, 960], BF16, kind="ExternalInput")
    wv_t = nc.dram_tensor("wv_p", [128, 984], BF16, kind="ExternalInput")
    wo_t = nc.dram_tensor("wo_p", [128, 960], BF16, kind="ExternalInput")
    sel_t = nc.dram_tensor("sel_p", [128, 32], BF16, kind="ExternalInput")
    bsel_t = nc.dram_tensor("bsel_p", [8, 384], BF16, kind="ExternalInput")
    rsel_t = nc.dram_tensor("rsel_p", [8, 384], F32R, kind="ExternalInput")
    bout_t = nc.dram_tensor("bout_p", [128, 3], F32, kind="ExternalInput")
    out_t = nc.dram_tensor("out", [B, INNER, HLOC, W], F32, kind="ExternalOutput")

    ctx_ap = ctx_t.ap()
    x_ap = x_t.ap()
    out_ap = out_t.ap()

    with tile.TileContext(nc) as tc, ExitStack() as ctxs:
        ep = ctxs.enter_context

        const_pool = ep(tc.tile_pool(name="const", bufs=1))
        ctx_pool = ep(tc.tile_pool(name="ctxp", bufs=9))
        x_pool = ep(tc.tile_pool(name="xp", bufs=2))
        q_pool = ep(tc.tile_pool(name="qp", bufs=2))
        tmp_pool = ep(tc.tile_pool(name="tmpp", bufs=10))
        vpd_pool = ep(tc.tile_pool(name="vpdp", bufs=6))
        s8_pool = ep(tc.tile_pool(name="s8p", bufs=2))
        ebc_pool = ep(tc.tile_pool(name="ebcp", bufs=4))
        mv_pool = ep(tc.tile_pool(name="mvp", bufs=3))
        fs_pool = ep(tc.tile_pool(name="fsp", bufs=2))
        sm_pool = ep(tc.tile_pool(name="smp", bufs=4))
        y_pool = ep(tc.tile_pool(name="yp", bufs=2))

        kps_pool = ep(tc.tile_pool(name="kps", bufs=3, space="PSUM"))
        vps_pool = ep(tc.tile_pool(name="vps", bufs=3, space="PSUM"))
        sps_pool = ep(tc.tile_pool(name="sps", bufs=2, space="PSUM"))

        # ---- constants ----
        wq_sb = const_pool.tile([128, 960], BF16, tag="wq")
        wk_sb = const_pool.tile([128, 960], BF16, tag="wk")
        wv_sb = const_pool.tile([128, 984], BF16, tag="wv")
        wo_sb = const_pool.tile([128, 960], BF16, tag="wo")
        sel_sb = const_pool.tile([128, 32], BF16, tag="sel")
        rsel_sb = const_pool.tile([128, 384], F32R, tag="rsel")
        bout_sb = const_pool.tile([128, 3], F32, tag="bout")
        for sb, dr in ((wq_sb, wq_t), (wk_sb, wk_t), (wv_sb, wv_t),
                       (wo_sb, wo_t), (sel_sb, sel_t),
                       (bout_sb, bout_t)):
            nc.sync.dma_start(sb[:], dr.ap())
        nc.sync.dma_start(rsel_sb[64:72, :], rsel_t.ap())

        def phase_a(blk):
            """DMA in, q projection, k projection, k*q, selector-reduce,
            drain sim to sbuf. Returns (ctx_sb, s8_sb)."""
            ctx_sb = []
            for kc, (c0, csz) in enumerate(CHUNKS):
                t = ctx_pool.tile([128, D * P], BF16, tag="ctx")
                ksz = csz + 1 if kc == 2 else csz  # chunk2 + ones-channel
                nc.sync.dma_start(t[0:ksz, :], ctx_ap[blk, c0:c0 + ksz, :])
                ctx_sb.append(t)
            x_sb = x_pool.tile([128, 384], BF16, tag="x")
            for kc, (c0, csz) in enumerate(CHUNKS):
                nc.sync.dma_start(x_sb[0:csz, kc * P:kc * P + P],
                                  x_ap[blk, c0:c0 + csz, :])

            q_ps = kps_pool.tile([128, 512], F32, tag="kp")
            for mo, (o0, osz) in enumerate(CHUNKS):
                for kc, (c0, csz) in enumerate(CHUNKS):
                    nc.tensor.matmul(
                        q_ps[0:osz, mo * P:mo * P + P],
                        wq_sb[0:csz, kc * 320 + o0:kc * 320 + o0 + osz],
                        x_sb[0:csz, kc * P:kc * P + P],
                        start=(kc == 0), stop=(kc == 2),
                    )
            # duplicate of chunk2 q at partition base 64 (col-tiled), used by
            # the odd-nt half of the paired chunk2 k*q
            for kc, (c0, csz) in enumerate(CHUNKS):
                nc.tensor.matmul(
                    q_ps[64:128, 2 * P:2 * P + P],
                    wq_sb[0:csz, kc * 320 + 256:kc * 320 + 256 + 64],
                    x_sb[0:csz, kc * P:kc * P + P],
                    start=(kc == 0), stop=(kc == 2),
                )
            q_sb = q_pool.tile([128, 384], BF16, tag="q")
            for mo, (o0, osz) in enumerate(CHUNKS):
                rows = 128 if mo == 2 else osz
                nc.scalar.activation(q_sb[0:rows, mo * P:mo * P + P],
                                     q_ps[0:rows, mo * P:mo * P + P],
                                     mybir.ActivationFunctionType.Copy)

            # k free layout: (pix, d) pixel-major; 512 = 16 pix x 32 d
            s8_sb = s8_pool.tile([8, D * P], BF16, tag="s8")

            def emit_k_pair(p):
                """nt pair (2p, 2p+1); chunk2 (64-wide) matmuls of the two
                nt are column-paired into one PSUM tile via tile_position
                so they run concurrently on separate PE column groups."""
                nt0, nt1 = 2 * p, 2 * p + 1
                tmps = {}
                kp_f = {}
                for j, nt in ((0, nt0), (1, nt1)):
                    for mo in (0, 1):
                        o0, osz = CHUNKS[mo]
                        kp = kps_pool.tile([128, 512], F32, tag="kp")
                        for kc, (c0, csz) in enumerate(CHUNKS):
                            nc.tensor.matmul(
                                kp[0:osz, :],
                                wk_sb[0:csz, kc * 320 + o0:kc * 320 + o0 + osz],
                                ctx_sb[kc][0:csz, nt * 512:(nt + 1) * 512],
                                start=(kc == 0), stop=(kc == 2),
                            )
                        kp_f[(j, mo)] = kp
                kp2 = kps_pool.tile([128, 512], F32, tag="kp")
                for kc, (c0, csz) in enumerate(CHUNKS):
                    for j, nt in ((0, nt0), (1, nt1)):
                        nc.tensor.matmul(
                            kp2[64 * j:64 * j + 64, :],
                            wk_sb[0:csz, kc * 320 + 256:kc * 320 + 256 + 64],
                            ctx_sb[kc][0:csz, nt * 512:(nt + 1) * 512],
                            start=(kc == 0), stop=(kc == 2),
                        )
                for j, nt in ((0, nt0), (1, nt1)):
                    for mo in (0, 1):
                        o0, osz = CHUNKS[mo]
                        tmp_t = tmp_pool.tile([128, 512], BF16, tag="tmp")
                        qb = q_sb[0:osz, mo * P + nt * 16:mo * P + nt * 16 + 16]
                        qb = qb.unsqueeze(2).to_broadcast((osz, 16, D))
                        nc.vector.tensor_mul(
                            tmp_t[0:osz, :].rearrange("c (x d) -> c x d", x=16),
                            kp_f[(j, mo)][0:osz, :].rearrange(
                                "c (x d) -> c x d", x=16),
                            qb,
                        )
                        tmps[(j, mo)] = tmp_t
                tmp2 = tmp_pool.tile([128, 512], BF16, tag="tmp")
                for j, nt in ((0, nt0), (1, nt1)):
                    b = 64 * j
                    qb = q_sb[b:b + 64, 2 * P + nt * 16:2 * P + nt * 16 + 16]
                    qb = qb.unsqueeze(2).to_broadcast((64, 16, D))
                    nc.vector.tensor_mul(
                        tmp2[b:b + 64, :].rearrange("c (x d) -> c x d", x=16),
                        kp2[b:b + 64, :].rearrange("c (x d) -> c x d", x=16),
                        qb,
                    )
                tmps["t2"] = tmp2
                return tmps

            def emit_sel_pair(p, tmps):
                nt0, nt1 = 2 * p, 2 * p + 1
                sims = []
                for j, nt in ((0, nt0), (1, nt1)):
                    sim_ps = sps_pool.tile([8, 512], F32, tag="sp",
                                           name=f"sim{j}")
                    for mo in (0, 1):
                        osz = CHUNKS[mo][1]
                        nc.tensor.matmul(
                            sim_ps[0:8, :],
                            sel_sb[0:osz, mo * 8:mo * 8 + 8],
                            tmps[(j, mo)][0:osz, :],
                            start=(mo == 0), stop=False,
                        )
                    sims.append(sim_ps)
                # chunk2 contributions row-paired (row groups 0 / 64)
                nc.tensor.matmul(sims[0][0:8, :], sel_sb[0:64, 16:24],
                                 tmps["t2"][0:64, :], start=False, stop=True)
                nc.tensor.matmul(sims[1][0:8, :], sel_sb[64:128, 24:32],
                                 tmps["t2"][64:128, :], start=False, stop=True)
                for j, nt in ((0, nt0), (1, nt1)):
                    # drain sim with exp fused (softmax numerator)
                    nc.scalar.activation(s8_sb[0:8, nt * 512:(nt + 1) * 512],
                                         sims[j][0:8, :],
                                         mybir.ActivationFunctionType.Exp)

            # sel(pair-1) emitted after k(pair): PE never waits on the DVE k*q
            prev = None
            for pr in range(NT // 2):
                cur = emit_k_pair(pr)
                if prev is not None:
                    emit_sel_pair(pr - 1, prev)
                prev = cur
            emit_sel_pair(NT // 2 - 1, prev)

            # broadcast attn rows 8 -> 320 via DRAM bounce: store s8, then
            # replicated reads (step-0 DRAM APs) land directly in SBUF as
            # per-chunk panels; chunk2 rows 64:72 carry raw attn for the
            # ones-channel denominator.
            # issue from the Scalar queue: these DMAs are gated on compute,
            # and would head-of-line-block the next block's ctx loads on Sync
            sc = s8d_t.ap()[blk % 2]
            nc.gpsimd.dma_start(sc, s8_sb[0:8, :])
            ebc_sb = [ebc_pool.tile([128, D * P], BF16, tag="ebc",
                                    name=f"ebc{mo}")
                      for mo in range(3)]
            for mo, (o0, osz) in enumerate(CHUNKS):
                nv = 64 if mo == 2 else VSZ[mo]
                for (r0, h0, nh, reps) in _bcast_runs(o0, nv):
                    src = sc[h0:h0 + nh, :].unsqueeze(1).to_broadcast(
                        (nh, reps, D * P))
                    nc.gpsimd.dma_start(ebc_sb[mo][r0:r0 + nh * reps, :], src)
            nc.gpsimd.dma_start(ebc_sb[2][64:72, :], sc)  # raw attn for den
            return ctx_sb, ebc_sb

        def phase_b1(blk, ctx_sb, ebc_sb):
            """V projection, v*attn, d-reduce."""
            mv_sb = []
            for mo in range(3):
                osz = VSZ[mo]
                o0 = CHUNKS[mo][0]
                t = mv_pool.tile([128, D * P], BF16, tag="mv")
                for nt in range(NT):
                    vp = vps_pool.tile([128, 512], F32, tag="vp")
                    for kc, (c0, csz) in enumerate(CHUNKS):
                        ksz = csz + 1 if kc == 2 else csz
                        nc.tensor.matmul(
                            vp[0:osz, :],
                            wv_sb[0:ksz, kc * 328 + o0:kc * 328 + o0 + osz],
                            ctx_sb[kc][0:ksz, nt * 512:(nt + 1) * 512],
                            start=(kc == 0), stop=(kc == 2),
                        )
                    vpd = vpd_pool.tile([128, 512], BF16, tag="vpd")
                    nc.scalar.activation(vpd[0:osz, :], vp[0:osz, :],
                                         mybir.ActivationFunctionType.Copy)
                    nc.vector.tensor_mul(
                        t[0:osz, nt * 512:(nt + 1) * 512],
                        vpd[0:osz, :],
                        ebc_sb[mo][0:osz, nt * 512:(nt + 1) * 512],
                    )
                mv_sb.append(t)

            # d-reduce as a bf16 fold tree (tensor_reduce is 1x-only; folded
            # tensor_tensor adds run 2x and cost ~2.4K cycles vs 4.3K)
            ov_sb = sm_pool.tile([128, 384], F32, tag="ov")
            for mo in range(3):
                osz = VSZ[mo]
                fs = fs_pool.tile([128, 3840], BF16, tag="fs")
                src = mv_sb[mo][0:osz, :].rearrange("c (x d) -> c x d", d=D)
                offs = [0, 2048, 3072, 3584]
                cur = src
                w = D
                for step in range(4):
                    w //= 2
                    dst = fs[0:osz, offs[step]:offs[step] + P * w]
                    dstv = dst.rearrange("c (x d) -> c x d", d=w)
                    nc.vector.tensor_add(dstv, cur[:, :, 0:w], cur[:, :, w:2 * w])
                    cur = dstv
                nc.vector.tensor_add(
                    ov_sb[0:osz, mo * P:mo * P + P].rearrange("c (x d) -> c x d", d=1),
                    cur[:, :, 0:1], cur[:, :, 1:2])
            return ov_sb

        def phase_b2(blk, ov_sb):
            """Reciprocal, normalize, output projection, DMA out."""
            b = blk // (PIX_B // P)
            p0 = (blk % (PIX_B // P)) * P
            hr = p0 // W
            nh = P // W

            r8_sb = sm_pool.tile([128, P], F32R, tag="r8")
            with nc.allow_low_precision(reason="f32r reciprocal feeding matmul"):
                nc.vector.reciprocal(r8_sb[64:72, :], ov_sb[64:72, 2 * P:3 * P])
            att_sb = sm_pool.tile([128, 384], BF16, tag="att")
            for mo, (o0, osz) in enumerate(CHUNKS):
                r_ps = kps_pool.tile([128, 512], F32, tag="kp")
                nc.tensor.matmul(
                    r_ps[0:osz, 0:P],
                    rsel_sb[64:72, mo * 128:mo * 128 + osz],
                    r8_sb[64:72, :],
                )
                nc.vector.tensor_mul(
                    att_sb[0:osz, mo * P:mo * P + P],
                    ov_sb[0:osz, mo * P:mo * P + P],
                    r_ps[0:osz, 0:P],
                )

            y_ps = vps_pool.tile([128, 512], F32, tag="vp")
            for mo, (o0, osz) in enumerate(CHUNKS):
                for kc, (c0, csz) in enumerate(CHUNKS):
                    nc.tensor.matmul(
                        y_ps[0:osz, mo * P:mo * P + P],
                        wo_sb[0:csz, kc * 320 + o0:kc * 320 + o0 + osz],
                        att_sb[0:csz, kc * P:kc * P + P],
                        start=(kc == 0), stop=(kc == 2),
                    )
            y_sb = y_pool.tile([128, 384], F32, tag="y")
            for mo, (o0, osz) in enumerate(CHUNKS):
                nc.scalar.add(
                    y_sb[0:osz, mo * P:mo * P + P],
                    y_ps[0:osz, mo * P:mo * P + P],
                    bout_sb[0:osz, mo:mo + 1],
                )
            for mo, (o0, osz) in enumerate(CHUNKS):
                dst = out_ap[b, o0:o0 + osz, hr:hr + nh, :].rearrange(
                    "c h w -> c (h w)")
                nc.sync.dma_start(dst, y_sb[0:osz, mo * P:mo * P + P])

        # software pipeline, depth 3: A(i) | B2(i-2) | B1(i-1).  B2 is
        # emitted before B1 so its short DVE ops (recip/att) are queued
        # ahead of B1's serial fold chain — else the y-projection stalls
        # the PE ~6.6us per block waiting behind the folds.
        st_a, st_b = {}, {}
        for blk in range(NBLK + 2):
            if blk < NBLK:
                st_a[blk] = phase_a(blk)
            if blk >= 2:
                phase_b2(blk - 2, st_b.pop(blk - 2))
            if 1 <= blk <= NBLK:
                st_b[blk - 1] = phase_b1(blk - 1, *st_a.pop(blk - 1))

    nc.compile()
    return nc


_CACHED = {}


def _get_nc():
    if "nc" not in _CACHED:
        _CACHED["nc"] = build_nc()
    return _CACHED["nc"]


def make_core_inputs(x, context, wq, wk, wv, wout, bout):
    """Full inputs -> list of 8 per-core input dicts (host prep: shard,
    block, append ones-channel, cast to bf16)."""
    sel, bsel, rsel = make_constants()
    consts = {
        "wq_p": pack_weight_T(np.asarray(wq, np.float32) * SCALE),
        "wk_p": pack_weight_T(np.asarray(wk, np.float32)),
        "wv_p": pack_weight_T(np.asarray(wv, np.float32), ones_cols=True),
        "wo_p": pack_weight_T(np.asarray(wout, np.float32)),
        "sel_p": sel, "bsel_p": bsel, "rsel_p": rsel,
    }
    bout_p = np.zeros((128, 3), np.float32)
    for mo, (o0, osz) in enumerate(CHUNKS):
        bout_p[0:osz, mo] = np.asarray(bout, np.float32)[o0:o0 + osz]
    consts["bout_p"] = bout_p
    x = np.asarray(x, np.float32)
    context = np.asarray(context, np.float32)
    nbh = PIX_B // P  # 4
    in_maps = []
    for cid in range(NCORES):
        h0 = cid * HLOC
        cs = context[:, :, :, h0:h0 + HLOC, :]  # [B, C, D, HLOC, W]
        cs = cs.reshape(B, CIN, D, nbh, P).transpose(0, 3, 1, 4, 2)
        cs = cs.reshape(NBLK, CIN, D * P)  # free = (pix, d), d innermost
        panel = np.ones((NBLK, CIN + 1, D * P), NPBF)
        panel[:, 0:CIN, :] = cs.astype(NPBF)
        xs = x[:, :, h0:h0 + HLOC, :].reshape(B, CIN, nbh, P).transpose(0, 2, 1, 3)
        xs = np.ascontiguousarray(xs.reshape(NBLK, CIN, P), dtype=NPBF)
        m = dict(consts)
        m["ctx"] = panel
        m["x"] = xs
        in_maps.append(m)
    return in_maps


def kernel(x, context, wq, wk, wv, wout, bout):
    from concourse.bass_utils import run_bass_kernel_spmd

    nc = _get_nc()
    in_maps = make_core_inputs(x, context, wq, wk, wv, wout, bout)
    res = run_bass_kernel_spmd(nc, in_maps, list(range(NCORES)))
    shards = [res.results[c]["out"] for c in range(NCORES)]
    return np.concatenate(shards, axis=2).astype(np.float32)


if __name__ == "__main__":
    nc = build_nc()
    print("build + compile OK")



# revision 36
# speedup vs baseline: 1.0810x; 1.0810x over previous
"""Trainium2 Bass kernel for nn_DepthAttention (depth attention over d=32).

Reference computation (per pixel (b,h,w), all 1x1 convs):
  q = Wq x               [320]   (heads=8 x dh=40)
  k = Wk ctx[:, d]       [320, 32]
  v = Wv ctx[:, d]       [320, 32]
  sim[n,d] = sum_{c in head n} q[c] k[c,d] * scale
  attn = softmax_d(sim)
  o[c] = sum_d v[c,d] attn[head(c),d]
  y = Wout o + bout      [320]

Sharding: h (64) split across 8 cores -> 8 rows of h per core, no halo.
Per core: 1024 pixels in 8 blocks of P=128. The host pre-casts
context/x to bf16 and lays them out as per-block contiguous panels
(one 8 KiB DMA run per channel row). The context panel carries a 321st
constant-ones channel: the v-projection's chunk-2 matmul uses it to
emit softmax-denominator rows for free.

The block loop is software-pipelined depth 3: phase A (DMA, q/k
projections, k*q, selector-reduce to sim[8 rows]) of block i is
emitted before phase B1 (broadcast+exp, v projection, v*attn,
d-reduce) of block i-1 and phase B2 (reciprocal, normalize, output
projection, DMA out) of block i-2, so the PE's in-order stream never
waits on the cross-engine softmax chain (which kept re-triggering the
HAM clock throttle in the depth-2 version).

Free-dim layout is pixel-major (p, d) — d innermost — so the d-reduce
on DVE reads stride-1.

Engine mapping:
  PE : all matmuls in bf16 (1 cyc/row, FWL weight loads): q/k/v
       projections, 320->8 selector reduce of k*q, 8->320 broadcast of
       sim, 1/den broadcast, Wout.
  DVE: k*q (q broadcast over d via step-0 AP), v*attn, reduce over d
       (d innermost via pixel-major v layout), reciprocal, bias add.
  ACT: psum->sbuf drains; exp AFTER broadcast on 128-row tiles.
       Softmax max-subtraction is skipped (logits are O(1) here).
"""

import sys

sys.path.insert(0, "/opt/trn_rl_repo")

from contextlib import ExitStack  # noqa: E402

import ml_dtypes  # noqa: E402
import numpy as np  # noqa: E402

import concourse.bacc as bacc  # noqa: E402
import concourse.bass as bass  # noqa: E402
import concourse.mybir as mybir  # noqa: E402
import concourse.tile as tile  # noqa: E402

HEADS = 8
DH = 40
CIN = 320
INNER = HEADS * DH  # 320
D = 32
B = 2
H = 64
W = 64
NCORES = 8
HLOC = H // NCORES  # 8
PIX_B = HLOC * W  # 512
P = 128
NBLK = B * PIX_B // P  # 8
NT = (D * P) // 512  # 8
SCALE = DH ** -0.5

F32 = mybir.dt.float32
F32R = mybir.dt.float32r
BF16 = mybir.dt.bfloat16
NPBF = ml_dtypes.bfloat16

CHUNKS = [(0, 128), (128, 128), (256, 64)]
# v-projection output sizes: chunk2 carries 8 extra denominator rows
VSZ = [128, 128, 72]


def _head_of(c):
    return c // DH


def _bcast_runs(o0, nv):
    """Row-replication runs for broadcasting attn head-rows into a chunk's
    channel rows: list of (row0, head0, n_heads, reps_per_head)."""
    runs, r = [], 0
    while r < nv:
        c = o0 + r
        h = c // DH
        run = min((h + 1) * DH - c, nv - r)
        if run == DH:
            nh = 1
            while (r + (nh + 1) * DH <= nv and (o0 + r + nh * DH) % DH == 0):
                nh += 1
            runs.append((r, h, nh, DH))
            r += nh * DH
        else:
            runs.append((r, h, 1, run))
            r += run
    return runs


def make_constants():
    sel = np.zeros((128, 32), NPBF)
    for kc, (c0, csz) in enumerate(CHUNKS):
        for r in range(csz):
            sel[r, kc * 8 + _head_of(c0 + r)] = 1.0
    for r in range(64):  # chunk2 selector replica at partition base 64
        sel[64 + r, 24 + _head_of(256 + r)] = 1.0
    bsel = np.zeros((8, 384), NPBF)
    for mo, (c0, csz) in enumerate(CHUNKS):
        for r in range(csz):
            bsel[_head_of(c0 + r), mo * 128 + r] = 1.0
    for n in range(8):
        bsel[n, 2 * 128 + 64 + n] = 1.0
    rsel = np.zeros((8, 384), np.float32)
    for mo, (c0, csz) in enumerate(CHUNKS):
        for r in range(csz):
            rsel[_head_of(c0 + r), mo * 128 + r] = 1.0
    return sel, bsel, rsel


def pack_weight_T(w, ones_cols=False):
    """w [out, in] -> bf16 packed lhsT [128, 3*M] with M = out (+8 den
    cols when ones_cols).  Chunk kc of the 'in' dim at free offset kc*M;
    chunk 2 gets an extra contraction row 64 (the ones-channel), wired to
    the 8 denominator columns when ones_cols."""
    wt = np.ascontiguousarray(w.T, dtype=np.float32)  # [in, out]
    od = wt.shape[1]
    m = od + 8 if ones_cols else od
    p = np.zeros((128, 3 * m), NPBF)
    for kc, (c0, csz) in enumerate(CHUNKS):
        p[0:csz, kc * m:kc * m + od] = wt[c0:c0 + csz, :]
    if ones_cols:
        for n in range(8):
            p[64, 2 * m + od + n] = 1.0  # ones-channel -> den col n (chunk2)
    return p


def build_nc():
    nc = bacc.Bacc(
        "TRN2",
        target_bir_lowering=False,
        debug=False,
        enable_asserts=False,
        num_devices=NCORES,
    )

    ctx_t = nc.dram_tensor("ctx", [NBLK, CIN + 1, D * P], BF16, kind="ExternalInput")
    s8d_t = nc.dram_tensor("s8d", [2, 8, D * P], BF16, kind="Internal")
    x_t = nc.dram_tensor("x", [NBLK, CIN, P], BF16, kind="ExternalInput")
    wq_t = nc.dram_tensor("wq_p", [128, 960], BF16, kind="ExternalInput")
    wk_t = nc.dram_tensor("wk_p", [128, 960], BF16, kind="ExternalInput")
    wv_t = nc.dram_tensor("wv_p", [128, 984], BF16, kind="ExternalInput")
    wo_t = nc.dram_tensor("wo_p", [128, 960], BF16, kind="ExternalInput")
    sel_t = nc.dram_tensor("sel_p", [128, 32], BF16, kind="ExternalInput")
    bsel_t = nc.dram_tensor("bsel_p", [8, 384], BF16, kind="ExternalInput")
    rsel_t = nc.dram_tensor("rsel_p", [8, 384], F32R, kind="ExternalInput")
    bout_t = nc.dram_tensor("bout_p", [128, 3], F32, kind="ExternalInput")
    out_t = nc.dram_tensor("out", [B, INNER, HLOC, W], F32, kind="ExternalOutput")

    ctx_ap = ctx_t.ap()
    x_ap = x_t.ap()
    out_ap = out_t.ap()

    with tile.TileContext(nc) as tc, ExitStack() as ctxs:
        ep = ctxs.enter_context

        const_pool = ep(tc.tile_pool(name="const", bufs=1))
        ctx_pool = ep(tc.tile_pool(name="ctxp", bufs=9))
        x_pool = ep(tc.tile_pool(name="xp", bufs=2))
        q_pool = ep(tc.tile_pool(name="qp", bufs=2))
        tmp_pool = ep(tc.tile_pool(name="tmpp", bufs=6))
        vpd_pool = ep(tc.tile_pool(name="vpdp", bufs=6))
        s8_pool = ep(tc.tile_pool(name="s8p", bufs=2))
        ebc_pool = ep(tc.tile_pool(name="ebcp", bufs=5))
        mv_pool = ep(tc.tile_pool(name="mvp", bufs=3))
        fs_pool = ep(tc.tile_pool(name="fsp", bufs=2))
        sm_pool = ep(tc.tile_pool(name="smp", bufs=4))
        y_pool = ep(tc.tile_pool(name="yp", bufs=2))

        kps_pool = ep(tc.tile_pool(name="kps", bufs=3, space="PSUM"))
        vps_pool = ep(tc.tile_pool(name="vps", bufs=3, space="PSUM"))
        sps_pool = ep(tc.tile_pool(name="sps", bufs=2, space="PSUM"))

        # ---- constants ----
        wq_sb = const_pool.tile([128, 960], BF16, tag="wq")
        wk_sb = const_pool.tile([128, 960], BF16, tag="wk")
        wv_sb = const_pool.tile([128, 984], BF16, tag="wv")
        wo_sb = const_pool.tile([128, 960], BF16, tag="wo")
        sel_sb = const_pool.tile([128, 32], BF16, tag="sel")
        rsel_sb = const_pool.tile([128, 384], F32R, tag="rsel")
        bout_sb = const_pool.tile([128, 3], F32, tag="bout")
        for sb, dr in ((wq_sb, wq_t), (wk_sb, wk_t), (wv_sb, wv_t),
                       (wo_sb, wo_t), (sel_sb, sel_t),
                       (bout_sb, bout_t)):
            nc.sync.dma_start(sb[:], dr.ap())
        nc.sync.dma_start(rsel_sb[64:72, :], rsel_t.ap())

        def phase_a(blk):
            """DMA in, q projection, k projection, k*q, selector-reduce,
            drain sim to sbuf. Returns (ctx_sb, s8_sb)."""
            ctx_sb = []
            for kc, (c0, csz) in enumerate(CHUNKS):
                t = ctx_pool.tile([128, D * P], BF16, tag="ctx")
                ksz = csz + 1 if kc == 2 else csz  # chunk2 + ones-channel
                nc.sync.dma_start(t[0:ksz, :], ctx_ap[blk, c0:c0 + ksz, :])
                ctx_sb.append(t)
            x_sb = x_pool.tile([128, 384], BF16, tag="x")
            for kc, (c0, csz) in enumerate(CHUNKS):
                nc.sync.dma_start(x_sb[0:csz, kc * P:kc * P + P],
                                  x_ap[blk, c0:c0 + csz, :])

            q_ps = kps_pool.tile([128, 512], F32, tag="kp")
            for mo, (o0, osz) in enumerate(CHUNKS):
                for kc, (c0, csz) in enumerate(CHUNKS):
                    nc.tensor.matmul(
                        q_ps[0:osz, mo * P:mo * P + P],
                        wq_sb[0:csz, kc * 320 + o0:kc * 320 + o0 + osz],
                        x_sb[0:csz, kc * P:kc * P + P],
                        start=(kc == 0), stop=(kc == 2),
                    )
            q_sb = q_pool.tile([128, 384], BF16, tag="q")
            for mo, (o0, osz) in enumerate(CHUNKS):
                nc.scalar.activation(q_sb[0:osz, mo * P:mo * P + P],
                                     q_ps[0:osz, mo * P:mo * P + P],
                                     mybir.ActivationFunctionType.Copy)

            # k free layout: (pix, d) pixel-major; 512 = 16 pix x 32 d
            s8_sb = s8_pool.tile([8, D * P], BF16, tag="s8")

            def emit_k(nt):
                tmp_ts = []
                for mo, (o0, osz) in enumerate(CHUNKS):
                    kp = kps_pool.tile([128, 512], F32, tag="kp")
                    for kc, (c0, csz) in enumerate(CHUNKS):
                        nc.tensor.matmul(
                            kp[0:osz, :],
                            wk_sb[0:csz, kc * 320 + o0:kc * 320 + o0 + osz],
                            ctx_sb[kc][0:csz, nt * 512:(nt + 1) * 512],
                            start=(kc == 0), stop=(kc == 2),
                        )
                    tmp_t = tmp_pool.tile([128, 512], BF16, tag="tmp")
                    qb = q_sb[0:osz, mo * P + nt * 16:mo * P + nt * 16 + 16]
                    qb = qb.unsqueeze(2).to_broadcast((osz, 16, D))
                    nc.vector.tensor_mul(
                        tmp_t[0:osz, :].rearrange("c (x d) -> c x d", x=16),
                        kp[0:osz, :].rearrange("c (x d) -> c x d", x=16),
                        qb,
                    )
                    tmp_ts.append(tmp_t)
                return tmp_ts

            def emit_sel(nt, tmp_ts):
                sim_ps = sps_pool.tile([8, 512], F32, tag="sp")
                for mo, (o0, osz) in enumerate(CHUNKS):
                    nc.tensor.matmul(
                        sim_ps[0:8, :],
                        sel_sb[0:osz, mo * 8:mo * 8 + 8],
                        tmp_ts[mo][0:osz, :],
                        start=(mo == 0), stop=(mo == 2),
                    )
                # drain sim with exp fused (softmax numerator, pre-broadcast)
                nc.scalar.activation(s8_sb[0:8, nt * 512:(nt + 1) * 512],
                                     sim_ps[0:8, :],
                                     mybir.ActivationFunctionType.Exp)

            # sel(nt-1) emitted after k(nt): PE never waits on the DVE k*q
            prev = None
            for nt in range(NT):
                cur = emit_k(nt)
                if prev is not None:
                    emit_sel(nt - 1, prev)
                prev = cur
            emit_sel(NT - 1, prev)

            # broadcast attn rows 8 -> 320 via DRAM bounce: store s8, then
            # replicated reads (step-0 DRAM APs) land directly in SBUF as
            # per-chunk panels; chunk2 rows 64:72 carry raw attn for the
            # ones-channel denominator.
            # issue from the Scalar queue: these DMAs are gated on compute,
            # and would head-of-line-block the next block's ctx loads on Sync
            sc = s8d_t.ap()[blk % 2]
            nc.gpsimd.dma_start(sc, s8_sb[0:8, :])
            ebc_sb = [ebc_pool.tile([128, D * P], BF16, tag="ebc",
                                    name=f"ebc{mo}")
                      for mo in range(3)]
            for mo, (o0, osz) in enumerate(CHUNKS):
                nv = 64 if mo == 2 else VSZ[mo]
                for (r0, h0, nh, reps) in _bcast_runs(o0, nv):
                    src = sc[h0:h0 + nh, :].unsqueeze(1).to_broadcast(
                        (nh, reps, D * P))
                    nc.gpsimd.dma_start(ebc_sb[mo][r0:r0 + nh * reps, :], src)
            nc.gpsimd.dma_start(ebc_sb[2][64:72, :], sc)  # raw attn for den
            return ctx_sb, ebc_sb

        def phase_b1(blk, ctx_sb, ebc_sb):
            """V projection, v*attn, d-reduce."""
            mv_sb = []
            for mo in range(3):
                osz = VSZ[mo]
                o0 = CHUNKS[mo][0]
                t = mv_pool.tile([128, D * P], BF16, tag="mv")
                for nt in range(NT):
                    vp = vps_pool.tile([128, 512], F32, tag="vp")
                    for kc, (c0, csz) in enumerate(CHUNKS):
                        ksz = csz + 1 if kc == 2 else csz
                        nc.tensor.matmul(
                            vp[0:osz, :],
                            wv_sb[0:ksz, kc * 328 + o0:kc * 328 + o0 + osz],
                            ctx_sb[kc][0:ksz, nt * 512:(nt + 1) * 512],
                            start=(kc == 0), stop=(kc == 2),
                        )
                    vpd = vpd_pool.tile([128, 512], BF16, tag="vpd")
                    nc.scalar.activation(vpd[0:osz, :], vp[0:osz, :],
                                         mybir.ActivationFunctionType.Copy)
                    nc.vector.tensor_mul(
                        t[0:osz, nt * 512:(nt + 1) * 512],
                        vpd[0:osz, :],
                        ebc_sb[mo][0:osz, nt * 512:(nt + 1) * 512],
                    )
                mv_sb.append(t)

            # d-reduce as a bf16 fold tree (tensor_reduce is 1x-only; folded
            # tensor_tensor adds run 2x and cost ~2.4K cycles vs 4.3K)
            ov_sb = sm_pool.tile([128, 384], F32, tag="ov")
            for mo in range(3):
                osz = VSZ[mo]
                fs = fs_pool.tile([128, 3840], BF16, tag="fs")
                src = mv_sb[mo][0:osz, :].rearrange("c (x d) -> c x d", d=D)
                offs = [0, 2048, 3072, 3584]
                cur = src
                w = D
                for step in range(4):
                    w //= 2
                    dst = fs[0:osz, offs[step]:offs[step] + P * w]
                    dstv = dst.rearrange("c (x d) -> c x d", d=w)
                    nc.vector.tensor_add(dstv, cur[:, :, 0:w], cur[:, :, w:2 * w])
                    cur = dstv
                nc.vector.tensor_add(
                    ov_sb[0:osz, mo * P:mo * P + P].rearrange("c (x d) -> c x d", d=1),
                    cur[:, :, 0:1], cur[:, :, 1:2])
            return ov_sb

        def phase_b2(blk, ov_sb):
            """Reciprocal, normalize, output projection, DMA out."""
            b = blk // (PIX_B // P)
            p0 = (blk % (PIX_B // P)) * P
            hr = p0 // W
            nh = P // W

            r8_sb = sm_pool.tile([128, P], F32R, tag="r8")
            with nc.allow_low_precision(reason="f32r reciprocal feeding matmul"):
                nc.vector.reciprocal(r8_sb[64:72, :], ov_sb[64:72, 2 * P:3 * P])
            att_sb = sm_pool.tile([128, 384], BF16, tag="att")
            for mo, (o0, osz) in enumerate(CHUNKS):
                r_ps = kps_pool.tile([128, 512], F32, tag="kp")
                nc.tensor.matmul(
                    r_ps[0:osz, 0:P],
                    rsel_sb[64:72, mo * 128:mo * 128 + osz],
                    r8_sb[64:72, :],
                )
                nc.vector.tensor_mul(
                    att_sb[0:osz, mo * P:mo * P + P],
                    ov_sb[0:osz, mo * P:mo * P + P],
                    r_ps[0:osz, 0:P],
                )

            y_ps = vps_pool.tile([128, 512], F32, tag="vp")
            for mo, (o0, osz) in enumerate(CHUNKS):
                for kc, (c0, csz) in enumerate(CHUNKS):
                    nc.tensor.matmul(
                        y_ps[0:osz, mo * P:mo * P + P],
                        wo_sb[0:csz, kc * 320 + o0:kc * 320 + o0 + osz],
                        att_sb[0:csz, kc * P:kc * P + P],
                        start=(kc == 0), stop=(kc == 2),
                    )
            y_sb = y_pool.tile([128, 384], F32, tag="y")
            for mo, (o0, osz) in enumerate(CHUNKS):
                nc.scalar.add(
                    y_sb[0:osz, mo * P:mo * P + P],
                    y_ps[0:osz, mo * P:mo * P + P],
                    bout_sb[0:osz, mo:mo + 1],
                )
            for mo, (o0, osz) in enumerate(CHUNKS):
                dst = out_ap[b, o0:o0 + osz, hr:hr + nh, :].rearrange(
                    "c h w -> c (h w)")
                nc.sync.dma_start(dst, y_sb[0:osz, mo * P:mo * P + P])

        # software pipeline, depth 3: A(i) | B2(i-2) | B1(i-1).  B2 is
        # emitted before B1 so its short DVE ops (recip/att) are queued
        # ahead of B1's serial fold chain — else the y-projection stalls
        # the PE ~6.6us per block waiting behind the folds.
        st_a, st_b = {}, {}
        for blk in range(NBLK + 2):
            if blk < NBLK:
                st_a[blk] = phase_a(blk)
            if blk >= 2:
                phase_b2(blk - 2, st_b.pop(blk - 2))
            if 1 <= blk <= NBLK:
                st_b[blk - 1] = phase_b1(blk - 1, *st_a.pop(blk - 1))

    nc.compile()
    return nc


_CACHED = {}


def _get_nc():
    if "nc" not in _CACHED:
        _CACHED["nc"] = build_nc()
    return _CACHED["nc"]


def make_core_inputs(x, context, wq, wk, wv, wout, bout):
    """Full inputs -> list of 8 per-core input dicts (host prep: shard,
    block, append ones-channel, cast to bf16)."""
    sel, bsel, rsel = make_constants()
    consts = {
        "wq_p": pack_weight_T(np.asarray(wq, np.float32) * SCALE),
        "wk_p": pack_weight_T(np.asarray(wk, np.float32)),
        "wv_p": pack_weight_T(np.asarray(wv, np.float32), ones_cols=True),
        "wo_p": pack_weight_T(np.asarray(wout, np.float32)),
        "sel_p": sel, "bsel_p": bsel, "rsel_p": rsel,
    }
    bout_p = np.zeros((128, 3), np.float32)
    for mo, (o0, osz) in enumerate(CHUNKS):
        bout_p[0:osz, mo] = np.asarray(bout, np.float32)[o0:o0 + osz]
    consts["bout_p"] = bout_p
    x = np.asarray(x, np.float32)
    context = np.asarray(context, np.float32)
    nbh = PIX_B // P  # 4
    in_maps = []
    for cid in range(NCORES):
        h0 = cid * HLOC
        cs = context[:, :, :, h0:h0 + HLOC, :]  # [B, C, D, HLOC, W]
        cs = cs.reshape(B, CIN, D, nbh, P).transpose(0, 3, 1, 4, 2)
        cs = cs.reshape(NBLK, CIN, D * P)  # free = (pix, d), d innermost
        panel = np.ones((NBLK, CIN + 1, D * P), NPBF)
        panel[:, 0:CIN, :] = cs.astype(NPBF)
        xs = x[:, :, h0:h0 + HLOC, :].reshape(B, CIN, nbh, P).transpose(0, 2, 1, 3)
        xs = np.ascontiguousarray(xs.reshape(NBLK, CIN, P), dtype=NPBF)
        m = dict(consts)
        m["ctx"] = panel
        m["x"] = xs
        in_maps.append(m)
    return in_maps


def kernel(x, context, wq, wk, wv, wout, bout):
    from concourse.bass_utils import run_bass_kernel_spmd

    nc = _get_nc()
    in_maps = make_core_inputs(x, context, wq, wk, wv, wout, bout)
    res = run_bass_kernel_spmd(nc, in_maps, list(range(NCORES)))
    shards = [res.results[c]["out"] for c in range(NCORES)]
    return np.concatenate(shards, axis=2).astype(np.float32)


if __name__ == "__main__":
    nc = build_nc()
    print("build + compile OK")



# revision 38
# speedup vs baseline: 1.1280x; 1.0435x over previous
"""Trainium2 Bass kernel for nn_DepthAttention (depth attention over d=32).

Reference computation (per pixel (b,h,w), all 1x1 convs):
  q = Wq x               [320]   (heads=8 x dh=40)
  k = Wk ctx[:, d]       [320, 32]
  v = Wv ctx[:, d]       [320, 32]
  sim[n,d] = sum_{c in head n} q[c] k[c,d] * scale
  attn = softmax_d(sim)
  o[c] = sum_d v[c,d] attn[head(c),d]
  y = Wout o + bout      [320]

Sharding: h (64) split across 8 cores -> 8 rows of h per core, no halo.
Per core: 1024 pixels in 8 blocks of P=128. The host pre-casts
context/x to bf16 and lays them out as per-block contiguous panels
(one 8 KiB DMA run per channel row). The context panel carries a 321st
constant-ones channel: the v-projection's chunk-2 matmul uses it to
emit softmax-denominator rows for free.

The block loop is software-pipelined depth 3, emitted as
A(i) | B2(i-2) | B1(i-1):
  A : ctx/x DMA, q proj, k proj, k*q (DVE, from PSUM), selector-reduce
      to sim[8 rows], ACT drain fused with exp, then the attn broadcast
      8->320 rows as DMA: s8 is bounced to a DRAM scratch and re-read
      with step-0 (replicating) source APs straight into per-chunk SBUF
      panels.  sel(nt-1) is emitted after k(nt) so the PE never waits
      on the DVE k*q.
  B1: v proj, ACT psum drain to bf16, v*attn (DVE 2x mode, both
      operands SBUF bf16), then the d-reduce as a log2 fold tree of
      tensor_tensor adds (tensor_reduce is capped at 1x mode).
  B2: reciprocal, 1/den broadcast matmul, normalize, Wout, bias (ACT),
      DMA out.  B2 is emitted before B1 so its short DVE ops queue
      ahead of B1's fold chain (else the y-projection stalls the PE
      ~6.6us per block, re-triggering the HAM clock throttle).

Free-dim layout is pixel-major (p, d) — d innermost — so the d-folds
read stride-1.  The broadcast/scratch DMAs are issued from the GpSimd
queue: they are gated on compute and would head-of-line-block the next
block's ctx loads (Sync queue) or the vp drains (Scalar queue).

Softmax max-subtraction is skipped (logits are O(1) here).  The
denominator comes for free: a constant ones-channel in the context
panel makes the v-projection's chunk-2 emit rows 64:72 = raw attn,
whose d-fold is sum_d exp(sim).
"""

import sys

sys.path.insert(0, "/opt/trn_rl_repo")

from contextlib import ExitStack  # noqa: E402

import ml_dtypes  # noqa: E402
import numpy as np  # noqa: E402

import concourse.bacc as bacc  # noqa: E402
import concourse.bass as bass  # noqa: E402
import concourse.mybir as mybir  # noqa: E402
import concourse.tile as tile  # noqa: E402

HEADS = 8
DH = 40
CIN = 320
INNER = HEADS * DH  # 320
D = 32
B = 2
H = 64
W = 64
NCORES = 8
HLOC = H // NCORES  # 8
PIX_B = HLOC * W  # 512
P = 128
NBLK = B * PIX_B // P  # 8
NT = (D * P) // 512  # 8
SCALE = DH ** -0.5

F32 = mybir.dt.float32
F32R = mybir.dt.float32r
BF16 = mybir.dt.bfloat16
NPBF = ml_dtypes.bfloat16

CHUNKS = [(0, 128), (128, 128), (256, 64)]
# v-projection output sizes: chunk2 carries 8 extra denominator rows
VSZ = [128, 128, 72]


def _head_of(c):
    return c // DH


def _bcast_runs(o0, nv):
    """Row-replication runs for broadcasting attn head-rows into a chunk's
    channel rows: list of (row0, head0, n_heads, reps_per_head)."""
    runs, r = [], 0
    while r < nv:
        c = o0 + r
        h = c // DH
        run = min((h + 1) * DH - c, nv - r)
        if run == DH:
            nh = 1
            while (r + (nh + 1) * DH <= nv and (o0 + r + nh * DH) % DH == 0):
                nh += 1
            runs.append((r, h, nh, DH))
            r += nh * DH
        else:
            runs.append((r, h, 1, run))
            r += run
    return runs


def make_constants():
    sel = np.zeros((128, 32), NPBF)
    for kc, (c0, csz) in enumerate(CHUNKS):
        for r in range(csz):
            sel[r, kc * 8 + _head_of(c0 + r)] = 1.0
    for r in range(64):  # chunk2 selector replica at partition base 64
        sel[64 + r, 24 + _head_of(256 + r)] = 1.0
    bsel = np.zeros((8, 384), NPBF)
    for mo, (c0, csz) in enumerate(CHUNKS):
        for r in range(csz):
            bsel[_head_of(c0 + r), mo * 128 + r] = 1.0
    for n in range(8):
        bsel[n, 2 * 128 + 64 + n] = 1.0
    rsel = np.zeros((8, 384), np.float32)
    for mo, (c0, csz) in enumerate(CHUNKS):
        for r in range(csz):
            rsel[_head_of(c0 + r), mo * 128 + r] = 1.0
    return sel, bsel, rsel


def pack_weight_T(w, ones_cols=False):
    """w [out, in] -> bf16 packed lhsT [128, 3*M] with M = out (+8 den
    cols when ones_cols).  Chunk kc of the 'in' dim at free offset kc*M;
    chunk 2 gets an extra contraction row 64 (the ones-channel), wired to
    the 8 denominator columns when ones_cols."""
    wt = np.ascontiguousarray(w.T, dtype=np.float32)  # [in, out]
    od = wt.shape[1]
    m = od + 8 if ones_cols else od
    p = np.zeros((128, 3 * m), NPBF)
    for kc, (c0, csz) in enumerate(CHUNKS):
        p[0:csz, kc * m:kc * m + od] = wt[c0:c0 + csz, :]
    if ones_cols:
        for n in range(8):
            p[64, 2 * m + od + n] = 1.0  # ones-channel -> den col n (chunk2)
    return p


def build_nc():
    nc = bacc.Bacc(
        "TRN2",
        target_bir_lowering=False,
        debug=False,
        enable_asserts=False,
        num_devices=NCORES,
    )

    ctx_t = nc.dram_tensor("ctx", [NBLK, CIN + 1, D * P], BF16, kind="ExternalInput")
    s8d_t = nc.dram_tensor("s8d", [2, 8, D * P], BF16, kind="Internal")
    x_t = nc.dram_tensor("x", [NBLK, CIN, P], BF16, kind="ExternalInput")
    wq_t = nc.dram_tensor("wq_p", [128, 960], BF16, kind="ExternalInput")
    wk_t = nc.dram_tensor("wk_p", [128, 960], BF16, kind="ExternalInput")
    wv_t = nc.dram_tensor("wv_p", [128, 984], BF16, kind="ExternalInput")
    wo_t = nc.dram_tensor("wo_p", [128, 960], BF16, kind="ExternalInput")
    sel_t = nc.dram_tensor("sel_p", [128, 32], BF16, kind="ExternalInput")
    bsel_t = nc.dram_tensor("bsel_p", [8, 384], BF16, kind="ExternalInput")
    rsel_t = nc.dram_tensor("rsel_p", [8, 384], F32R, kind="ExternalInput")
    bout_t = nc.dram_tensor("bout_p", [128, 3], F32, kind="ExternalInput")
    out_t = nc.dram_tensor("out", [B, INNER, HLOC, W], F32, kind="ExternalOutput")

    ctx_ap = ctx_t.ap()
    x_ap = x_t.ap()
    out_ap = out_t.ap()

    with tile.TileContext(nc) as tc, ExitStack() as ctxs:
        ep = ctxs.enter_context

        const_pool = ep(tc.tile_pool(name="const", bufs=1))
        ctx_pool = ep(tc.tile_pool(name="ctxp", bufs=9))
        x_pool = ep(tc.tile_pool(name="xp", bufs=2))
        q_pool = ep(tc.tile_pool(name="qp", bufs=2))
        tmp_pool = ep(tc.tile_pool(name="tmpp", bufs=6))
        vpd_pool = ep(tc.tile_pool(name="vpdp", bufs=6))
        s8_pool = ep(tc.tile_pool(name="s8p", bufs=2))
        ebc_pool = ep(tc.tile_pool(name="ebcp", bufs=5))
        mv_pool = ep(tc.tile_pool(name="mvp", bufs=3))
        fs_pool = ep(tc.tile_pool(name="fsp", bufs=2))
        sm_pool = ep(tc.tile_pool(name="smp", bufs=4))
        y_pool = ep(tc.tile_pool(name="yp", bufs=2))

        kps_pool = ep(tc.tile_pool(name="kps", bufs=3, space="PSUM"))
        vps_pool = ep(tc.tile_pool(name="vps", bufs=3, space="PSUM"))
        sps_pool = ep(tc.tile_pool(name="sps", bufs=2, space="PSUM"))

        # ---- constants ----
        wq_sb = const_pool.tile([128, 960], BF16, tag="wq")
        wk_sb = const_pool.tile([128, 960], BF16, tag="wk")
        wv_sb = const_pool.tile([128, 984], BF16, tag="wv")
        wo_sb = const_pool.tile([128, 960], BF16, tag="wo")
        sel_sb = const_pool.tile([128, 32], BF16, tag="sel")
        rsel_sb = const_pool.tile([128, 384], F32R, tag="rsel")
        bout_sb = const_pool.tile([128, 3], F32, tag="bout")
        for sb, dr in ((wq_sb, wq_t), (wk_sb, wk_t), (wv_sb, wv_t),
                       (wo_sb, wo_t), (sel_sb, sel_t),
                       (bout_sb, bout_t)):
            nc.sync.dma_start(sb[:], dr.ap())
        nc.sync.dma_start(rsel_sb[64:72, :], rsel_t.ap())

        def phase_a(blk):
            """DMA in, q projection, k projection, k*q, selector-reduce,
            drain sim to sbuf. Returns (ctx_sb, s8_sb)."""
            ctx_sb = []
            for kc, (c0, csz) in enumerate(CHUNKS):
                t = ctx_pool.tile([128, D * P], BF16, tag="ctx")
                ksz = csz + 1 if kc == 2 else csz  # chunk2 + ones-channel
                nc.sync.dma_start(t[0:ksz, :], ctx_ap[blk, c0:c0 + ksz, :])
                ctx_sb.append(t)
            x_sb = x_pool.tile([128, 384], BF16, tag="x")
            for kc, (c0, csz) in enumerate(CHUNKS):
                nc.sync.dma_start(x_sb[0:csz, kc * P:kc * P + P],
                                  x_ap[blk, c0:c0 + csz, :])

            q_ps = kps_pool.tile([128, 512], F32, tag="kp")
            for mo, (o0, osz) in enumerate(CHUNKS):
                for kc, (c0, csz) in enumerate(CHUNKS):
                    nc.tensor.matmul(
                        q_ps[0:osz, mo * P:mo * P + P],
                        wq_sb[0:csz, kc * 320 + o0:kc * 320 + o0 + osz],
                        x_sb[0:csz, kc * P:kc * P + P],
                        start=(kc == 0), stop=(kc == 2),
                    )
            q_sb = q_pool.tile([128, 384], BF16, tag="q")
            for mo, (o0, osz) in enumerate(CHUNKS):
                nc.scalar.activation(q_sb[0:osz, mo * P:mo * P + P],
                                     q_ps[0:osz, mo * P:mo * P + P],
                                     mybir.ActivationFunctionType.Copy)

            # k free layout: (pix, d) pixel-major; 512 = 16 pix x 32 d
            s8_sb = s8_pool.tile([8, D * P], BF16, tag="s8")

            def emit_k(nt):
                tmp_ts = []
                for mo, (o0, osz) in enumerate(CHUNKS):
                    kp = kps_pool.tile([128, 512], F32, tag="kp")
                    for kc, (c0, csz) in enumerate(CHUNKS):
                        nc.tensor.matmul(
                            kp[0:osz, :],
                            wk_sb[0:csz, kc * 320 + o0:kc * 320 + o0 + osz],
                            ctx_sb[kc][0:csz, nt * 512:(nt + 1) * 512],
                            start=(kc == 0), stop=(kc == 2),
                        )
                    tmp_t = tmp_pool.tile([128, 512], BF16, tag="tmp")
                    qb = q_sb[0:osz, mo * P + nt * 16:mo * P + nt * 16 + 16]
                    qb = qb.unsqueeze(2).to_broadcast((osz, 16, D))
                    nc.vector.tensor_mul(
                        tmp_t[0:osz, :].rearrange("c (x d) -> c x d", x=16),
                        kp[0:osz, :].rearrange("c (x d) -> c x d", x=16),
                        qb,
                    )
                    tmp_ts.append(tmp_t)
                return tmp_ts

            def emit_sel(nt, tmp_ts):
                sim_ps = sps_pool.tile([8, 512], F32, tag="sp")
                for mo, (o0, osz) in enumerate(CHUNKS):
                    nc.tensor.matmul(
                        sim_ps[0:8, :],
                        sel_sb[0:osz, mo * 8:mo * 8 + 8],
                        tmp_ts[mo][0:osz, :],
                        start=(mo == 0), stop=(mo == 2),
                    )
                # drain sim with exp fused (softmax numerator, pre-broadcast)
                nc.scalar.activation(s8_sb[0:8, nt * 512:(nt + 1) * 512],
                                     sim_ps[0:8, :],
                                     mybir.ActivationFunctionType.Exp)

            # sel(nt-1) emitted after k(nt): PE never waits on the DVE k*q
            prev = None
            for nt in range(NT):
                cur = emit_k(nt)
                if prev is not None:
                    emit_sel(nt - 1, prev)
                prev = cur
            emit_sel(NT - 1, prev)

            # broadcast attn rows 8 -> 320 via DRAM bounce: store s8, then
            # replicated reads (step-0 DRAM APs) land directly in SBUF as
            # per-chunk panels; chunk2 rows 64:72 carry raw attn for the
            # ones-channel denominator.
            # issue from the Scalar queue: these DMAs are gated on compute,
            # and would head-of-line-block the next block's ctx loads on Sync
            sc = s8d_t.ap()[blk % 2]
            nc.gpsimd.dma_start(sc, s8_sb[0:8, :])
            ebc_sb = [ebc_pool.tile([128, D * P], BF16, tag="ebc",
                                    name=f"ebc{mo}")
                      for mo in range(3)]
            for mo, (o0, osz) in enumerate(CHUNKS):
                nv = 64 if mo == 2 else VSZ[mo]
                for (r0, h0, nh, reps) in _bcast_runs(o0, nv):
                    src = sc[h0:h0 + nh, :].unsqueeze(1).to_broadcast(
                        (nh, reps, D * P))
                    nc.gpsimd.dma_start(ebc_sb[mo][r0:r0 + nh * reps, :], src)
            nc.gpsimd.dma_start(ebc_sb[2][64:72, :], sc)  # raw attn for den
            return ctx_sb, ebc_sb

        def phase_b1(blk, ctx_sb, ebc_sb):
            """V projection, v*attn, d-reduce."""
            mv_sb = []
            for mo in range(3):
                osz = VSZ[mo]
                o0 = CHUNKS[mo][0]
                t = mv_pool.tile([128, D * P], BF16, tag="mv")
                for nt in range(NT):
                    vp = vps_pool.tile([128, 512], F32, tag="vp")
                    for kc, (c0, csz) in enumerate(CHUNKS):
                        ksz = csz + 1 if kc == 2 else csz
                        nc.tensor.matmul(
                            vp[0:osz, :],
                            wv_sb[0:ksz, kc * 328 + o0:kc * 328 + o0 + osz],
                            ctx_sb[kc][0:ksz, nt * 512:(nt + 1) * 512],
                            start=(kc == 0), stop=(kc == 2),
                        )
                    vpd = vpd_pool.tile([128, 512], BF16, tag="vpd")
                    nc.scalar.activation(vpd[0:osz, :], vp[0:osz, :],
                                         mybir.ActivationFunctionType.Copy)
                    nc.vector.tensor_mul(
                        t[0:osz, nt * 512:(nt + 1) * 512],
                        vpd[0:osz, :],
                        ebc_sb[mo][0:osz, nt * 512:(nt + 1) * 512],
                    )
                mv_sb.append(t)

            # d-reduce as a bf16 fold tree (tensor_reduce is 1x-only; folded
            # tensor_tensor adds run 2x and cost ~2.4K cycles vs 4.3K)
            ov_sb = sm_pool.tile([128, 384], F32, tag="ov")
            for mo in range(3):
                osz = VSZ[mo]
                fs = fs_pool.tile([128, 3840], BF16, tag="fs")
                src = mv_sb[mo][0:osz, :].rearrange("c (x d) -> c x d", d=D)
                offs = [0, 2048, 3072, 3584]
                cur = src
                w = D
                for step in range(4):
                    w //= 2
                    dst = fs[0:osz, offs[step]:offs[step] + P * w]
                    dstv = dst.rearrange("c (x d) -> c x d", d=w)
                    nc.vector.tensor_add(dstv, cur[:, :, 0:w], cur[:, :, w:2 * w])
                    cur = dstv
                nc.vector.tensor_add(
                    ov_sb[0:osz, mo * P:mo * P + P].rearrange("c (x d) -> c x d", d=1),
                    cur[:, :, 0:1], cur[:, :, 1:2])
            return ov_sb

        def phase_b2(blk, ov_sb):
            """Reciprocal, normalize, output projection, DMA out."""
            b = blk // (PIX_B // P)
            p0 = (blk % (PIX_B // P)) * P
            hr = p0 // W
            nh = P // W

            r8_sb = sm_pool.tile([128, P], F32R, tag="r8")
            with nc.allow_low_precision(reason="f32r reciprocal feeding matmul"):
                nc.vector.reciprocal(r8_sb[64:72, :], ov_sb[64:72, 2 * P:3 * P])
            att_sb = sm_pool.tile([128, 384], BF16, tag="att")
            r_ps = kps_pool.tile([128, 512], F32, tag="kp")
            for mo, (o0, osz) in enumerate(CHUNKS):
                nc.tensor.matmul(
                    r_ps[0:osz, mo * P:mo * P + P],
                    rsel_sb[64:72, mo * 128:mo * 128 + osz],
                    r8_sb[64:72, :],
                )
            # one normalize multiply over all three chunks; rows past each
            # chunk's VSZ are junk x junk and never read by the y-projection
            nc.vector.tensor_mul(
                att_sb[0:128, 0:384],
                ov_sb[0:128, 0:384],
                r_ps[0:128, 0:384],
            )

            y_ps = vps_pool.tile([128, 512], F32, tag="vp")
            for mo, (o0, osz) in enumerate(CHUNKS):
                for kc, (c0, csz) in enumerate(CHUNKS):
                    nc.tensor.matmul(
                        y_ps[0:osz, mo * P:mo * P + P],
                        wo_sb[0:csz, kc * 320 + o0:kc * 320 + o0 + osz],
                        att_sb[0:csz, kc * P:kc * P + P],
                        start=(kc == 0), stop=(kc == 2),
                    )
            y_sb = y_pool.tile([128, 384], F32, tag="y")
            for mo, (o0, osz) in enumerate(CHUNKS):
                nc.scalar.add(
                    y_sb[0:osz, mo * P:mo * P + P],
                    y_ps[0:osz, mo * P:mo * P + P],
                    bout_sb[0:osz, mo:mo + 1],
                )
            for mo, (o0, osz) in enumerate(CHUNKS):
                dst = out_ap[b, o0:o0 + osz, hr:hr + nh, :].rearrange(
                    "c h w -> c (h w)")
                nc.sync.dma_start(dst, y_sb[0:osz, mo * P:mo * P + P])

        # software pipeline, depth 3: A(i) | B2(i-2) | B1(i-1).  B2 is
        # emitted before B1 so its short DVE ops (recip/att) are queued
        # ahead of B1's serial fold chain — else the y-projection stalls
        # the PE ~6.6us per block waiting behind the folds.
        st_a, st_b = {}, {}
        for blk in range(NBLK + 2):
            if blk < NBLK:
                st_a[blk] = phase_a(blk)
            if blk >= 2:
                phase_b2(blk - 2, st_b.pop(blk - 2))
            if 1 <= blk <= NBLK:
                st_b[blk - 1] = phase_b1(blk - 1, *st_a.pop(blk - 1))

    nc.compile()
    return nc


_CACHED = {}


def _get_nc():
    if "nc" not in _CACHED:
        _CACHED["nc"] = build_nc()
    return _CACHED["nc"]


def make_core_inputs(x, context, wq, wk, wv, wout, bout):
    """Full inputs -> list of 8 per-core input dicts (host prep: shard,
    block, append ones-channel, cast to bf16)."""
    sel, bsel, rsel = make_constants()
    consts = {
        "wq_p": pack_weight_T(np.asarray(wq, np.float32) * SCALE),
        "wk_p": pack_weight_T(np.asarray(wk, np.float32)),
        "wv_p": pack_weight_T(np.asarray(wv, np.float32), ones_cols=True),
        "wo_p": pack_weight_T(np.asarray(wout, np.float32)),
        "sel_p": sel, "bsel_p": bsel, "rsel_p": rsel,
    }
    bout_p = np.zeros((128, 3), np.float32)
    for mo, (o0, osz) in enumerate(CHUNKS):
        bout_p[0:osz, mo] = np.asarray(bout, np.float32)[o0:o0 + osz]
    consts["bout_p"] = bout_p
    x = np.asarray(x, np.float32)
    context = np.asarray(context, np.float32)
    nbh = PIX_B // P  # 4
    in_maps = []
    for cid in range(NCORES):
        h0 = cid * HLOC
        cs = context[:, :, :, h0:h0 + HLOC, :]  # [B, C, D, HLOC, W]
        cs = cs.reshape(B, CIN, D, nbh, P).transpose(0, 3, 1, 4, 2)
        cs = cs.reshape(NBLK, CIN, D * P)  # free = (pix, d), d innermost
        panel = np.ones((NBLK, CIN + 1, D * P), NPBF)
        panel[:, 0:CIN, :] = cs.astype(NPBF)
        xs = x[:, :, h0:h0 + HLOC, :].reshape(B, CIN, nbh, P).transpose(0, 2, 1, 3)
        xs = np.ascontiguousarray(xs.reshape(NBLK, CIN, P), dtype=NPBF)
        m = dict(consts)
        m["ctx"] = panel
        m["x"] = xs
        in_maps.append(m)
    return in_maps


def kernel(x, context, wq, wk, wv, wout, bout):
    from concourse.bass_utils import run_bass_kernel_spmd

    nc = _get_nc()
    in_maps = make_core_inputs(x, context, wq, wk, wv, wout, bout)
    res = run_bass_kernel_spmd(nc, in_maps, list(range(NCORES)))
    shards = [res.results[c]["out"] for c in range(NCORES)]
    return np.concatenate(shards, axis=2).astype(np.float32)


if __name__ == "__main__":
    nc = build_nc()
    print("build + compile OK")

